# revision 2
# baseline (speedup 1.0000x reference)
"""ConceptContrastiveLoss Trainium2 kernel (8-core SPMD, batch-parallel).

Takes FULL inputs expert_concepts/violator_concepts [256, 2048, 128] f32,
returns the scalar loss.  Internally shards the batch dim across 8 cores
(32 E + 32 V batch items = 64 MiB of input per core).

Bulk phase (per core, the memory-bound part): each batch item [2048, 128]
is DMA'd as one contiguous 1 MiB transfer into SBUF [128 part x 2048]
(16 seq rows per partition), alternating the two HWDGE rings (sync/
scalar).  The seq-reduction is split so no single engine becomes the
bottleneck (the v1 kernel's full fp32 tree-halve on DVE was ~168us busy
-- above the ~150-190us DMA stream time -- because fp32 tensor_tensor is
1x rate = (N+151)/0.96ns):
  - DVE does ONE halving level with a bf16 downcast:
    Th = Tb[:, :1024] + Tb[:, 1024:]  (fp32 in, bf16 out)
    => (1024+151)/0.96 ~ 1.22us/tile, ~78us total.
  - PE folds the 128 partitions with a one-hot bf16 stationary: for local
    batch c, lhsT = W[:, 128-c:192-c] where W has a single ones-column, so
    the column-sum of the moving tile lands in PSUM row c.  Two N=512 bf16
    matmuls per batch accumulate all 64 batches into one 2-bank PSUM tile
    (~0.5us/batch, ~35us total; stationary is data-independent so there is
    no fp32 LoadStationary bottleneck).
  - One DVE tensor_reduce folds the remaining 8 seq groups (j-fold), a PE
    transpose + ACT scale produce centS[d, c] = centroid/S.
Critical path is the DMA stream; measured (For_i loop-delta, interleaved
runs) the bulk runs at the pure-DMA floor (~150-175us/iter sustained vs
v1's ~195+), and bf16 intermediates keep |rel err| ~1e-6 on the loss.

Tail (~14us serialized, measured via unrolled-tails loop-delta; v1 was
~35us): AllGather the [128, 64] f32 local centroids (8-core AG floor
~5us), reload with core-major packing, cast/pack to bf16, then the
pairwise phase entirely in bf16: -2*X^T*Y cross matmuls + K=64 aug
matmul adding (n_i + n_j) via rows {0,32}, ACT sqrt/relu hinge + Square
with accum_out (NOT DVE tensor_tensor_reduce -- that instruction hangs
the device: NRT_EXEC_UNIT_UNRECOVERABLE), weighted combine, ones-matmul
partition fold, scalar out.
"""

import numpy as np
from contextlib import ExitStack

import concourse.bacc as bacc
import concourse.bass as bass
import concourse.mybir as mybir
import concourse.tile as tile
from concourse.bass_utils import run_bass_kernel_spmd
from concourse.masks import make_identity
from concourse.tile import add_dep_helper

F32 = mybir.dt.float32
BF16 = mybir.dt.bfloat16
AX = mybir.AxisListType
ALU = mybir.AluOpType
ACTF = mybir.ActivationFunctionType

MARGIN = 10.0
ALPHA = 3.0
BETA = 0.3
GAMMA = 0.3

B, S, D = 256, 2048, 128
N_CORES = 8

BPD = 1       # batch items per DMA
BUFS = 8      # big-tile pool buffers
N_DMA_ENG = 2  # HWDGE rings: nc.sync + nc.scalar (gpsimd SWDGE measured slower)


def _emit_body(tc, nc, e, v, out, loc_cent, gath, n_cores, *, bpd, bufs,
               n_dma_eng, loop_r, bulk, tail, solo, unroll_k, tails_only, ctx):
    Bl = B // n_cores          # local batches per tensor (32)
    nloc = 2 * Bl              # local batch columns (64)
    n_tiles = nloc // bpd
    n_pairs = B * (B - 1) // 2
    w_ev = ALPHA / (B * B)
    w_ee = BETA / (2.0 * n_pairs)
    w_vv = GAMMA / (2.0 * n_pairs)

    consts = ctx.enter_context(tc.tile_pool(name="consts", bufs=1))
    ones_f32 = nc.const_aps.aps[(F32, 1.0)]
    ones_bf16 = nc.const_aps.aps[(BF16, 1.0)]
    b_eps = consts.tile([128, 1], F32, name="b_eps")
    nc.vector.memset(b_eps[:], 1e-12)
    b_margin = consts.tile([128, 1], F32, name="b_margin")
    nc.vector.memset(b_margin[:], MARGIN)
    # one-hot window: W[:, 128] = 1, else 0.  lhsT = W[:, 128-c : 192-c]
    W = consts.tile([128, 256], BF16, name="Wonehot")
    nc.vector.memset(W[:], 0.0)
    nc.vector.memset(W[:, 128:129], 1.0)
    ident = consts.tile([128, 128], F32, name="ident")
    make_identity(nc, ident[:])
    centS = consts.tile([D, nloc], F32, name="centS")

    big = ctx.enter_context(tc.tile_pool(name="big", bufs=bufs))
    half = ctx.enter_context(tc.tile_pool(name="half", bufs=bufs))
    cps = ctx.enter_context(tc.tile_pool(name="cps", bufs=1, space="PSUM"))
    sp = ctx.enter_context(tc.tile_pool(name="sp", bufs=1))
    pps = ctx.enter_context(tc.tile_pool(name="pps", bufs=1, space="PSUM"))
    trash = ctx.enter_context(tc.tile_pool(name="trash", bufs=2))

    T01 = cps.tile([128, 1024], F32, name="T01")
    Cs = sp.tile([64, 128], F32, name="Cs")
    Tp = cps.tile([128, 512], F32, name="Tp")

    dma_engines = [nc.sync, nc.scalar, nc.gpsimd][:n_dma_eng]

    def emit_bulk():
        first_dmas = []
        dma_i = 0
        for t in range(n_tiles):
            gi0 = t * bpd
            t_idx, b0 = divmod(gi0, Bl)
            src = (e, v)[t_idx]
            Tb = big.tile([128, bpd * 2048], F32, name="Tb")
            eng = dma_engines[dma_i % len(dma_engines)]
            dma_i += 1
            if bpd == 1:
                dma = eng.dma_start(
                    out=Tb[:],
                    in_=src[b0].rearrange("(p j) d -> p (j d)", p=128),
                )
            else:
                dma = eng.dma_start(
                    out=Tb[:].rearrange("p (b x) -> p b x", b=bpd),
                    in_=src[b0:b0 + bpd].rearrange(
                        "b (p j) d -> p b (j d)", p=128
                    ),
                )
            if t < len(dma_engines):
                first_dmas.append(dma)
            if bulk == "nored":
                continue
            Th = half.tile([128, bpd * 1024], BF16, name="Th")
            if bpd == 1:
                nc.vector.tensor_add(Th[:], Tb[:, 0:1024], Tb[:, 1024:2048])
            else:
                Tb3 = Tb[:].rearrange("p (b x) -> p b x", b=bpd)
                Th3 = Th[:].rearrange("p (b x) -> p b x", b=bpd)
                nc.vector.tensor_add(
                    Th3[:, :, :], Tb3[:, :, 0:1024], Tb3[:, :, 1024:2048]
                )
            for bi in range(bpd):
                c = gi0 + bi
                lhsT = W[:, 128 - c:192 - c]
                off = bi * 1024
                first = c == 0
                last = c == nloc - 1
                nc.tensor.matmul(
                    out=T01[0:64, 0:512], lhsT=lhsT, rhs=Th[:, off:off + 512],
                    start=first, stop=last, skip_group_check=True,
                )
                nc.tensor.matmul(
                    out=T01[0:64, 512:1024], lhsT=lhsT,
                    rhs=Th[:, off + 512:off + 1024],
                    start=first, stop=last, skip_group_check=True,
                )
        if bulk == "nored":
            return first_dmas
        # finish: Cs[c, d] = sum_j T01[c, (j d)] ; centS[d, c] = Cs[c, d] / S
        nc.vector.tensor_reduce(
            out=Cs[:], in_=T01[0:64, :].rearrange("p (j d) -> p d j", d=128),
            axis=AX.X, op=ALU.add,
        )
        nc.tensor.transpose(Tp[:, 0:64], Cs[:], ident[0:64, 0:64])
        nc.scalar.mul(centS[:], Tp[:, 0:64], 1.0 / S)
        return first_dmas

    def emit_tail(k):
        if tail == "none":
            fin0 = sp.tile([1, 1], F32, name="fin0", tag="fin0")
            nc.vector.memset(fin0[:], 0.0)
            nc.sync.dma_start(out=loc_cent[:], in_=centS[:])
            return nc.sync.dma_start(out=out[:], in_=fin0[:])
        # ---- gather centroids across cores ----
        nc.sync.dma_start(out=loc_cent[:], in_=centS[:])
        if solo:
            nc.sync.dma_start(out=gath[0:D, :], in_=loc_cent[:])
        else:
            nc.gpsimd.collective_compute(
                "AllGather",
                ALU.bypass,
                replica_groups=[list(range(n_cores))],
                ins=[loc_cent[:]],
                outs=[gath[:]],
            )
        # reload: CtEV[p, (c m)] = gath[(c p), m]; cols = (core, [E32|V32])
        CtEV = sp.tile([D, 2 * B], F32, name="CtEV", tag="CtEV")
        nc.sync.dma_start(
            out=CtEV[:].rearrange("p (c m) -> p c m", c=n_cores),
            in_=gath.rearrange("(c p) m -> p c m", c=n_cores),
        )

        # un-interleave into contiguous bf16 tiles: cols [E(256) | V(256)]
        # (global batch order = (core, j), a permutation -- loss invariant).
        # matmul operand APs must be 2D, so pack via DVE copies.
        def tv(ap, t_idx):
            return ap.rearrange("p (c t j) -> p t c j", c=n_cores, t=2)[:, t_idx]

        Cb = sp.tile([D, 2 * B], BF16, name="Cb", tag="Cb")
        Cm2 = sp.tile([D, 2 * B], BF16, name="Cm2", tag="Cm2")
        sqb = sp.tile([D, 2 * B], BF16, name="sqb", tag="sqb")
        for ti in (0, 1):
            half_ = slice(ti * B, ti * B + B)
            out3 = Cb[:, half_].rearrange("p (c j) -> p c j", c=n_cores)
            nc.vector.tensor_copy(out3, tv(CtEV[:], ti))
            out3m = Cm2[:, half_].rearrange("p (c j) -> p c j", c=n_cores)
            nc.vector.tensor_scalar_mul(out3m, tv(CtEV[:], ti), -2.0)
        nc.vector.tensor_mul(sqb[:], Cb[:], Cb[:])
        psn_t = pps.tile([128, 512], F32, name="psn", tag="psn", bufs=1)
        nc.tensor.matmul(out=psn_t[0:1, :], lhsT=ones_bf16, rhs=sqb[:])
        nE = psn_t[0:1, 0:B]
        nV = psn_t[0:1, B:2 * B]

        # aug tiles: rows {0, 32} carry {norms, ones} (partition offsets must
        # be 32-aligned); the rest is zero, K=64 for the aug matmuls.
        A_e = sp.tile([64, B], BF16, name="A_e", tag="A_e")
        A_v = sp.tile([64, B], BF16, name="A_v", tag="A_v")
        R_e = sp.tile([64, B], BF16, name="R_e", tag="R_e")
        R_v = sp.tile([64, B], BF16, name="R_v", tag="R_v")
        for t_, row0, row32 in (
            (A_e, nE, None), (A_v, nV, None), (R_e, None, nE), (R_v, None, nV)
        ):
            nc.vector.memset(t_[:], 0.0)
            if row0 is not None:
                nc.vector.tensor_copy(t_[0:1, :], row0)
                nc.vector.memset(t_[32:33, :], 1.0)
            else:
                nc.vector.memset(t_[0:1, :], 1.0)
                nc.vector.tensor_copy(t_[32:33, :], row32)

        CbE, CbV = Cb[:, 0:B], Cb[:, B:2 * B]
        Cm2E, Cm2V = Cm2[:, 0:B], Cm2[:, B:2 * B]

        acc = sp.tile([128, 6], F32, name="acc", tag="acc")
        ci = 0
        # EV separation (2 row blocks) then EE/VV clustering (2 blocks each)
        for m2, Ct, Ag, Rt, is_ev in (
            (Cm2E, CbV, A_e, R_v, True),
            (Cm2E, CbE, A_e, R_e, False),
            (Cm2V, CbV, A_v, R_v, False),
        ):
            for blk in (0, 1):
                P_t = pps.tile([128, 512], F32, name="P", tag="P", bufs=2)
                P_ = P_t[:, 0:256]
                nc.tensor.matmul(
                    out=P_, lhsT=m2[:, 128 * blk:128 * blk + 128],
                    rhs=Ct, start=True, stop=False, skip_group_check=True,
                )
                nc.tensor.matmul(
                    out=P_, lhsT=Ag[:, 128 * blk:128 * blk + 128], rhs=Rt[:],
                    start=False, stop=True, skip_group_check=True,
                )
                if is_ev:
                    dist = trash.tile([128, B], F32, name="dist", tag="dist")
                    hin = trash.tile([128, B], F32, name="hin", tag="hin")
                    hsq = trash.tile([128, B], F32, name="hsq", tag="hsq")
                    nc.scalar.activation(
                        dist[:], P_, ACTF.Sqrt, bias=b_eps[:],
                    )
                    nc.scalar.activation(
                        hin[:], dist[:], ACTF.Relu, bias=b_margin[:],
                        scale=-1.0,
                    )
                    nc.scalar.activation(
                        hsq[:], hin[:], ACTF.Square, accum_out=acc[:, ci:ci + 1],
                    )
                else:
                    rel = trash.tile([128, B], F32, name="rel", tag="dist")
                    nc.scalar.activation(
                        rel[:], P_, ACTF.Relu, accum_out=acc[:, ci:ci + 1],
                    )
                ci += 1

        # combine: tot = w_ev*(acc0+acc1) + w_ee*(acc2+acc3) + w_vv*(acc4+acc5)
        t_ev = sp.tile([128, 1], F32, name="t_ev", tag="t_ev")
        t_ee = sp.tile([128, 1], F32, name="t_ee", tag="t_ee")
        t_vv = sp.tile([128, 1], F32, name="t_vv", tag="t_vv")
        tot = sp.tile([128, 1], F32, name="tot", tag="tot")
        nc.vector.tensor_add(t_ev[:], acc[:, 0:1], acc[:, 1:2])
        nc.vector.tensor_add(t_ee[:], acc[:, 2:3], acc[:, 3:4])
        nc.vector.tensor_add(t_vv[:], acc[:, 4:5], acc[:, 5:6])
        nc.vector.tensor_scalar_mul(tot[:], t_ev[:], w_ev)
        nc.vector.scalar_tensor_tensor(
            tot[:], t_ee[:], w_ee, tot[:], op0=ALU.mult, op1=ALU.add,
        )
        nc.vector.scalar_tensor_tensor(
            tot[:], t_vv[:], w_vv, tot[:], op0=ALU.mult, op1=ALU.add,
        )
        psF_t = pps.tile([128, 512], F32, name="psF", tag="psF", bufs=1)
        nc.tensor.matmul(out=psF_t[0:1, 0:1], lhsT=ones_f32, rhs=tot[:])
        fin = sp.tile([1, 1], F32, name="fin", tag="fin")
        nc.scalar.copy(fin[:], psF_t[0:1, 0:1])
        od = nc.sync.dma_start(out=out[:], in_=fin[:])
        return od

    if unroll_k > 1 and tails_only:
        emit_bulk()
        for k in range(unroll_k):
            emit_tail(k)
    elif unroll_k > 1:
        prev = None
        for k in range(unroll_k):
            fds = emit_bulk()
            if prev is not None:
                for d in fds:
                    add_dep_helper(d.ins, prev.ins, sync=True,
                                   reason="serialize e2e iterations")
            prev = emit_tail(k)
    else:
        if loop_r > 1:
            with tc.For_i(0, loop_r, 1) as _i:
                emit_bulk()
        else:
            emit_bulk()
        if bulk == "nored":
            nc.vector.memset(centS[:], 0.0)
        emit_tail(0)


def build_nc(n_cores=N_CORES, *, bpd=BPD, bufs=BUFS, n_dma_eng=N_DMA_ENG,
             loop_r=1, bulk="pemov", tail="full", solo=False, unroll_k=1,
             tails_only=False):
    Bl = B // n_cores
    nc = bacc.Bacc("TRN2", num_devices=n_cores)
    e = nc.dram_tensor("expert_concepts", [Bl, S, D], F32,
                       kind="ExternalInput").ap()
    v = nc.dram_tensor("violator_concepts", [Bl, S, D], F32,
                       kind="ExternalInput").ap()
    out = nc.dram_tensor("out", [1, 1], F32, kind="ExternalOutput").ap()
    loc_cent = nc.dram_tensor("loc_cent", [D, 2 * Bl], F32).ap()
    gath = nc.dram_tensor(
        "gath", [n_cores * D, 2 * Bl], F32,
        addr_space="Local" if solo else "Shared",
    ).ap()
    with tile.TileContext(nc) as tc:
        with ExitStack() as ctx:
            _emit_body(tc, nc, e, v, out, loc_cent, gath, n_cores,
                       bpd=bpd, bufs=bufs, n_dma_eng=n_dma_eng, loop_r=loop_r,
                       bulk=bulk, tail=tail, solo=solo, unroll_k=unroll_k,
                       tails_only=tails_only, ctx=ctx)
    nc.compile()
    return nc


def make_in_maps(expert_concepts, violator_concepts, n_cores=N_CORES):
    expert_concepts = np.ascontiguousarray(expert_concepts, dtype=np.float32)
    violator_concepts = np.ascontiguousarray(violator_concepts, dtype=np.float32)
    Bl = B // n_cores
    return [
        {
            "expert_concepts": expert_concepts[c * Bl:(c + 1) * Bl],
            "violator_concepts": violator_concepts[c * Bl:(c + 1) * Bl],
        }
        for c in range(n_cores)
    ]


def _run(expert_concepts, violator_concepts, **build_kwargs):
    assert expert_concepts.shape == (B, S, D)
    assert violator_concepts.shape == (B, S, D)
    nc = build_nc(**build_kwargs)
    in_maps = make_in_maps(expert_concepts, violator_concepts)
    res = run_bass_kernel_spmd(nc, in_maps, list(range(N_CORES)))
    return np.float32(res.results[0]["out"][0, 0]), res


def kernel(expert_concepts: np.ndarray, violator_concepts: np.ndarray) -> np.ndarray:
    out, _ = _run(expert_concepts, violator_concepts)
    return out


# revision 4
# speedup vs baseline: 1.4065x; 1.4065x over previous
"""ConceptContrastiveLoss Trainium2 kernel (8-core SPMD, batch-parallel).

Takes FULL inputs expert_concepts/violator_concepts [256, 2048, 128] f32,
returns the scalar loss.  Internally shards the batch dim across 8 cores
(32 E + 32 V batch items = 64 MiB of input per core).

Bulk phase (per core, the memory-bound part): each batch item [2048, 128]
is DMA'd as one contiguous 1 MiB transfer into SBUF [128 part x 2048]
(16 seq rows per partition), alternating the two HWDGE rings (sync/
scalar).  The seq-reduction is split so no single engine becomes the
bottleneck (the v1 kernel's full fp32 tree-halve on DVE was ~168us busy
-- above the ~150-190us DMA stream time -- because fp32 tensor_tensor is
1x rate = (N+151)/0.96ns):
  - DVE does ONE halving level with a bf16 downcast:
    Th = Tb[:, :1024] + Tb[:, 1024:]  (fp32 in, bf16 out)
    => (1024+151)/0.96 ~ 1.22us/tile, ~78us total.
  - PE folds the 128 partitions with a one-hot bf16 stationary: for local
    batch c, lhsT = W[:, 128-c:192-c] where W has a single ones-column, so
    the column-sum of the moving tile lands in PSUM row c.  Two N=512 bf16
    matmuls per batch accumulate all 64 batches into one 2-bank PSUM tile
    (~0.5us/batch, ~35us total; stationary is data-independent so there is
    no fp32 LoadStationary bottleneck).
  - One DVE tensor_reduce folds the remaining 8 seq groups (j-fold), a PE
    transpose + ACT scale produce centS[d, c] = centroid/S.
Critical path is the DMA stream; measured (For_i loop-delta, interleaved
runs) the bulk runs at the pure-DMA floor (~150-175us/iter sustained vs
v1's ~195+), and bf16 intermediates keep |rel err| ~1e-6 on the loss.

Tail (~14us serialized, measured via unrolled-tails loop-delta; v1 was
~35us): AllGather the [128, 64] f32 local centroids (8-core AG floor
~5us), reload with core-major packing, cast/pack to bf16, then the
pairwise phase entirely in bf16: -2*X^T*Y cross matmuls + K=64 aug
matmul adding (n_i + n_j) via rows {0,32}, ACT sqrt/relu hinge + Square
with accum_out (NOT DVE tensor_tensor_reduce -- that instruction hangs
the device: NRT_EXEC_UNIT_UNRECOVERABLE), weighted combine, ones-matmul
partition fold, scalar out.
"""

import numpy as np
from contextlib import ExitStack

import concourse.bacc as bacc
import concourse.bass as bass
import concourse.mybir as mybir
import concourse.tile as tile
from concourse.bass_utils import run_bass_kernel_spmd
from concourse.masks import make_identity
from concourse.tile import add_dep_helper

F32 = mybir.dt.float32
BF16 = mybir.dt.bfloat16
AX = mybir.AxisListType
ALU = mybir.AluOpType
ACTF = mybir.ActivationFunctionType

MARGIN = 10.0
ALPHA = 3.0
BETA = 0.3
GAMMA = 0.3

B, S, D = 256, 2048, 128
N_CORES = 8

BPD = 1       # batch items per DMA
BUFS = 8      # big-tile pool buffers
N_DMA_ENG = 2  # HWDGE rings: nc.sync + nc.scalar (gpsimd SWDGE measured slower)


def _emit_body(tc, nc, e, v, out, loc_cent, gath, n_cores, *, bpd, bufs,
               n_dma_eng, loop_r, bulk, tail, solo, unroll_k, tails_only, ctx):
    Bl = B // n_cores          # local batches per tensor (32)
    nloc = 2 * Bl              # local batch columns (64)
    n_tiles = nloc // bpd
    n_pairs = B * (B - 1) // 2
    w_ev = ALPHA / (B * B)
    w_ee = BETA / (2.0 * n_pairs)
    w_vv = GAMMA / (2.0 * n_pairs)

    consts = ctx.enter_context(tc.tile_pool(name="consts", bufs=1))
    ones_f32 = nc.const_aps.aps[(F32, 1.0)]
    ones_bf16 = nc.const_aps.aps[(BF16, 1.0)]
    b_eps = consts.tile([128, 1], F32, name="b_eps")
    nc.vector.memset(b_eps[:], 1e-12)
    b_margin = consts.tile([128, 1], F32, name="b_margin")
    nc.vector.memset(b_margin[:], MARGIN)
    # one-hot window: W[:, 128] = 1, else 0.  lhsT = W[:, 128-c : 192-c]
    W = consts.tile([128, 256], BF16, name="Wonehot")
    nc.vector.memset(W[:], 0.0)
    nc.vector.memset(W[:, 128:129], 1.0)
    ident = consts.tile([128, 128], F32, name="ident")
    make_identity(nc, ident[:])
    centS = consts.tile([D, nloc], F32, name="centS")

    big = ctx.enter_context(tc.tile_pool(name="big", bufs=bufs))
    half = ctx.enter_context(tc.tile_pool(name="half", bufs=bufs))
    cps = ctx.enter_context(tc.tile_pool(name="cps", bufs=1, space="PSUM"))
    sp = ctx.enter_context(tc.tile_pool(name="sp", bufs=1))
    pps = ctx.enter_context(tc.tile_pool(name="pps", bufs=1, space="PSUM"))
    trash = ctx.enter_context(tc.tile_pool(name="trash", bufs=2))

    T01 = cps.tile([128, 1024], F32, name="T01")
    Cs = sp.tile([64, 128], F32, name="Cs")
    Tp = cps.tile([128, 512], F32, name="Tp")

    dma_engines = [nc.sync, nc.scalar, nc.gpsimd][:n_dma_eng]

    def emit_bulk():
        first_dmas = []
        dma_i = 0
        for t in range(n_tiles):
            gi0 = t * bpd
            t_idx, b0 = divmod(gi0, Bl)
            src = (e, v)[t_idx]
            Tb = big.tile([128, bpd * 2048], F32, name="Tb")
            eng = dma_engines[dma_i % len(dma_engines)]
            dma_i += 1
            if bpd == 1:
                dma = eng.dma_start(
                    out=Tb[:],
                    in_=src[b0].rearrange("(p j) d -> p (j d)", p=128),
                )
            else:
                dma = eng.dma_start(
                    out=Tb[:].rearrange("p (b x) -> p b x", b=bpd),
                    in_=src[b0:b0 + bpd].rearrange(
                        "b (p j) d -> p b (j d)", p=128
                    ),
                )
            if t < len(dma_engines):
                first_dmas.append(dma)
            if bulk == "nored":
                continue
            Th = half.tile([128, bpd * 1024], BF16, name="Th")
            if bpd == 1:
                nc.vector.tensor_add(Th[:], Tb[:, 0:1024], Tb[:, 1024:2048])
            else:
                Tb3 = Tb[:].rearrange("p (b x) -> p b x", b=bpd)
                Th3 = Th[:].rearrange("p (b x) -> p b x", b=bpd)
                nc.vector.tensor_add(
                    Th3[:, :, :], Tb3[:, :, 0:1024], Tb3[:, :, 1024:2048]
                )
            for bi in range(bpd):
                c = gi0 + bi
                lhsT = W[:, 128 - c:192 - c]
                off = bi * 1024
                first = c == 0
                last = c == nloc - 1
                nc.tensor.matmul(
                    out=T01[0:64, 0:512], lhsT=lhsT, rhs=Th[:, off:off + 512],
                    start=first, stop=last, skip_group_check=True,
                )
                nc.tensor.matmul(
                    out=T01[0:64, 512:1024], lhsT=lhsT,
                    rhs=Th[:, off + 512:off + 1024],
                    start=first, stop=last, skip_group_check=True,
                )
        if bulk == "nored":
            return first_dmas
        # finish: Cs[c, d] = sum_j T01[c, (j d)] ; centS[d, c] = Cs[c, d] / S
        nc.vector.tensor_reduce(
            out=Cs[:], in_=T01[0:64, :].rearrange("p (j d) -> p d j", d=128),
            axis=AX.X, op=ALU.add,
        )
        nc.tensor.transpose(Tp[:, 0:64], Cs[:], ident[0:64, 0:64])
        nc.scalar.mul(centS[:], Tp[:, 0:64], 1.0 / S)
        return first_dmas

    def emit_tail(k):
        if tail == "none":
            fin0 = sp.tile([1, 1], F32, name="fin0", tag="fin0")
            nc.vector.memset(fin0[:], 0.0)
            nc.sync.dma_start(out=loc_cent[:], in_=centS[:])
            return nc.sync.dma_start(out=out[:], in_=fin0[:])
        # ---- gather centroids across cores ----
        nc.sync.dma_start(out=loc_cent[:], in_=centS[:])
        if solo:
            nc.sync.dma_start(out=gath[0:D, :], in_=loc_cent[:])
        else:
            nc.gpsimd.collective_compute(
                "AllGather",
                ALU.bypass,
                replica_groups=[list(range(n_cores))],
                ins=[loc_cent[:]],
                outs=[gath[:]],
            )
        # reload: CtEV[p, (c m)] = gath[(c p), m]; cols = (core, [E32|V32])
        CtEV = sp.tile([D, 2 * B], F32, name="CtEV", tag="CtEV")
        nc.sync.dma_start(
            out=CtEV[:].rearrange("p (c m) -> p c m", c=n_cores),
            in_=gath.rearrange("(c p) m -> p c m", c=n_cores),
        )

        # un-interleave into contiguous bf16 tiles: cols [E(256) | V(256)]
        # (global batch order = (core, j), a permutation -- loss invariant).
        # matmul operand APs must be 2D, so pack via DVE copies.
        def tv(ap, t_idx):
            return ap.rearrange("p (c t j) -> p t c j", c=n_cores, t=2)[:, t_idx]

        Cb = sp.tile([D, 2 * B], BF16, name="Cb", tag="Cb")
        Cm2 = sp.tile([D, 2 * B], BF16, name="Cm2", tag="Cm2")
        sqb = sp.tile([D, 2 * B], BF16, name="sqb", tag="sqb")
        for ti in (0, 1):
            half_ = slice(ti * B, ti * B + B)
            out3 = Cb[:, half_].rearrange("p (c j) -> p c j", c=n_cores)
            nc.vector.tensor_copy(out3, tv(CtEV[:], ti))
            out3m = Cm2[:, half_].rearrange("p (c j) -> p c j", c=n_cores)
            nc.vector.tensor_scalar_mul(out3m, tv(CtEV[:], ti), -2.0)
        nc.vector.tensor_mul(sqb[:], Cb[:], Cb[:])
        psn_t = pps.tile([128, 512], F32, name="psn", tag="psn", bufs=1)
        nc.tensor.matmul(out=psn_t[0:1, :], lhsT=ones_bf16, rhs=sqb[:])
        nE = psn_t[0:1, 0:B]
        nV = psn_t[0:1, B:2 * B]

        # aug tiles: rows {0, 32} carry {norms, ones} (partition offsets must
        # be 32-aligned); the rest is zero, K=64 for the aug matmuls.
        A_e = sp.tile([64, B], BF16, name="A_e", tag="A_e")
        A_v = sp.tile([64, B], BF16, name="A_v", tag="A_v")
        R_e = sp.tile([64, B], BF16, name="R_e", tag="R_e")
        R_v = sp.tile([64, B], BF16, name="R_v", tag="R_v")
        for t_, row0, row32 in (
            (A_e, nE, None), (A_v, nV, None), (R_e, None, nE), (R_v, None, nV)
        ):
            nc.vector.memset(t_[:], 0.0)
            if row0 is not None:
                nc.vector.tensor_copy(t_[0:1, :], row0)
                nc.vector.memset(t_[32:33, :], 1.0)
            else:
                nc.vector.memset(t_[0:1, :], 1.0)
                nc.vector.tensor_copy(t_[32:33, :], row32)

        CbE, CbV = Cb[:, 0:B], Cb[:, B:2 * B]
        Cm2E, Cm2V = Cm2[:, 0:B], Cm2[:, B:2 * B]

        acc = sp.tile([128, 6], F32, name="acc", tag="acc")
        # accum_out accumulates onto existing memory contents -- acc MUST be
        # zeroed (observed: stale SBUF garbage => nondeterministic wrong loss)
        nc.vector.memset(acc[:], 0.0)
        ci = 0
        # EV separation (2 row blocks) then EE/VV clustering (2 blocks each)
        for m2, Ct, Ag, Rt, is_ev in (
            (Cm2E, CbV, A_e, R_v, True),
            (Cm2E, CbE, A_e, R_e, False),
            (Cm2V, CbV, A_v, R_v, False),
        ):
            for blk in (0, 1):
                P_t = pps.tile([128, 512], F32, name="P", tag="P", bufs=2)
                P_ = P_t[:, 0:256]
                nc.tensor.matmul(
                    out=P_, lhsT=m2[:, 128 * blk:128 * blk + 128],
                    rhs=Ct, start=True, stop=False, skip_group_check=True,
                )
                nc.tensor.matmul(
                    out=P_, lhsT=Ag[:, 128 * blk:128 * blk + 128], rhs=Rt[:],
                    start=False, stop=True, skip_group_check=True,
                )
                if is_ev:
                    dist = trash.tile([128, B], F32, name="dist", tag="dist")
                    hin = trash.tile([128, B], F32, name="hin", tag="hin")
                    hsq = trash.tile([128, B], F32, name="hsq", tag="hsq")
                    nc.scalar.activation(
                        dist[:], P_, ACTF.Sqrt, bias=b_eps[:],
                    )
                    nc.scalar.activation(
                        hin[:], dist[:], ACTF.Relu, bias=b_margin[:],
                        scale=-1.0,
                    )
                    nc.scalar.activation(
                        hsq[:], hin[:], ACTF.Square, accum_out=acc[:, ci:ci + 1],
                    )
                else:
                    rel = trash.tile([128, B], F32, name="rel", tag="dist")
                    nc.scalar.activation(
                        rel[:], P_, ACTF.Relu, accum_out=acc[:, ci:ci + 1],
                    )
                ci += 1

        # combine: tot = w_ev*(acc0+acc1) + w_ee*(acc2+acc3) + w_vv*(acc4+acc5)
        t_ev = sp.tile([128, 1], F32, name="t_ev", tag="t_ev")
        t_ee = sp.tile([128, 1], F32, name="t_ee", tag="t_ee")
        t_vv = sp.tile([128, 1], F32, name="t_vv", tag="t_vv")
        tot = sp.tile([128, 1], F32, name="tot", tag="tot")
        nc.vector.tensor_add(t_ev[:], acc[:, 0:1], acc[:, 1:2])
        nc.vector.tensor_add(t_ee[:], acc[:, 2:3], acc[:, 3:4])
        nc.vector.tensor_add(t_vv[:], acc[:, 4:5], acc[:, 5:6])
        nc.vector.tensor_scalar_mul(tot[:], t_ev[:], w_ev)
        nc.vector.scalar_tensor_tensor(
            tot[:], t_ee[:], w_ee, tot[:], op0=ALU.mult, op1=ALU.add,
        )
        nc.vector.scalar_tensor_tensor(
            tot[:], t_vv[:], w_vv, tot[:], op0=ALU.mult, op1=ALU.add,
        )
        psF_t = pps.tile([128, 512], F32, name="psF", tag="psF", bufs=1)
        nc.tensor.matmul(out=psF_t[0:1, 0:1], lhsT=ones_f32, rhs=tot[:])
        fin = sp.tile([1, 1], F32, name="fin", tag="fin")
        nc.scalar.copy(fin[:], psF_t[0:1, 0:1])
        od = nc.sync.dma_start(out=out[:], in_=fin[:])
        return od

    if unroll_k > 1 and tails_only:
        emit_bulk()
        for k in range(unroll_k):
            emit_tail(k)
    elif unroll_k > 1:
        prev = None
        for k in range(unroll_k):
            fds = emit_bulk()
            if prev is not None:
                for d in fds:
                    add_dep_helper(d.ins, prev.ins, sync=True,
                                   reason="serialize e2e iterations")
            prev = emit_tail(k)
    else:
        if loop_r > 1:
            with tc.For_i(0, loop_r, 1) as _i:
                emit_bulk()
        else:
            emit_bulk()
        if bulk == "nored":
            nc.vector.memset(centS[:], 0.0)
        emit_tail(0)


def build_nc(n_cores=N_CORES, *, bpd=BPD, bufs=BUFS, n_dma_eng=N_DMA_ENG,
             loop_r=1, bulk="pemov", tail="full", solo=False, unroll_k=1,
             tails_only=False):
    Bl = B // n_cores
    nc = bacc.Bacc("TRN2", num_devices=n_cores)
    e = nc.dram_tensor("expert_concepts", [Bl, S, D], F32,
                       kind="ExternalInput").ap()
    v = nc.dram_tensor("violator_concepts", [Bl, S, D], F32,
                       kind="ExternalInput").ap()
    out = nc.dram_tensor("out", [1, 1], F32, kind="ExternalOutput").ap()
    loc_cent = nc.dram_tensor("loc_cent", [D, 2 * Bl], F32).ap()
    gath = nc.dram_tensor(
        "gath", [n_cores * D, 2 * Bl], F32,
        addr_space="Local" if solo else "Shared",
    ).ap()
    with tile.TileContext(nc) as tc:
        with ExitStack() as ctx:
            _emit_body(tc, nc, e, v, out, loc_cent, gath, n_cores,
                       bpd=bpd, bufs=bufs, n_dma_eng=n_dma_eng, loop_r=loop_r,
                       bulk=bulk, tail=tail, solo=solo, unroll_k=unroll_k,
                       tails_only=tails_only, ctx=ctx)
    nc.compile()
    return nc


def make_in_maps(expert_concepts, violator_concepts, n_cores=N_CORES):
    expert_concepts = np.ascontiguousarray(expert_concepts, dtype=np.float32)
    violator_concepts = np.ascontiguousarray(violator_concepts, dtype=np.float32)
    Bl = B // n_cores
    return [
        {
            "expert_concepts": expert_concepts[c * Bl:(c + 1) * Bl],
            "violator_concepts": violator_concepts[c * Bl:(c + 1) * Bl],
        }
        for c in range(n_cores)
    ]


def _run(expert_concepts, violator_concepts, **build_kwargs):
    assert expert_concepts.shape == (B, S, D)
    assert violator_concepts.shape == (B, S, D)
    nc = build_nc(**build_kwargs)
    in_maps = make_in_maps(expert_concepts, violator_concepts)
    res = run_bass_kernel_spmd(nc, in_maps, list(range(N_CORES)))
    return np.float32(res.results[0]["out"][0, 0]), res


def kernel(expert_concepts: np.ndarray, violator_concepts: np.ndarray) -> np.ndarray:
    out, _ = _run(expert_concepts, violator_concepts)
    return out


# revision 6
# speedup vs baseline: 1.5746x; 1.1195x over previous
"""ConceptContrastiveLoss Trainium2 kernel (8-core SPMD, batch-parallel).

Takes FULL inputs expert_concepts/violator_concepts [256, 2048, 128] f32,
returns the scalar loss.  Internally shards the batch dim across 8 cores
(32 E + 32 V batch items = 64 MiB of input per core).

Bulk phase (per core, the memory-bound part): each batch item [2048, 128]
is DMA'd as one contiguous 1 MiB transfer into SBUF [128 part x 2048]
(16 seq rows per partition), alternating the two HWDGE rings (sync/
scalar).  The seq-reduction is split so no single engine becomes the
bottleneck (the v1 kernel's full fp32 tree-halve on DVE was ~168us busy
-- above the ~150-190us DMA stream time -- because fp32 tensor_tensor is
1x rate = (N+151)/0.96ns):
  - DVE does ONE halving level with a bf16 downcast:
    Th = Tb[:, :1024] + Tb[:, 1024:]  (fp32 in, bf16 out)
    => (1024+151)/0.96 ~ 1.22us/tile, ~78us total.
  - PE folds the 128 partitions with a one-hot bf16 stationary: for local
    batch c, lhsT = W[:, 128-c:192-c] where W has a single ones-column, so
    the column-sum of the moving tile lands in PSUM row c.  Two N=512 bf16
    matmuls per batch accumulate all 64 batches into one 2-bank PSUM tile
    (~0.5us/batch, ~35us total; stationary is data-independent so there is
    no fp32 LoadStationary bottleneck).
  - One DVE tensor_reduce folds the remaining 8 seq groups (j-fold), a PE
    transpose + ACT scale produce centS[d, c] = centroid/S.
Critical path is the DMA stream; measured (For_i loop-delta, interleaved
runs) the bulk runs at the pure-DMA floor (~150-175us/iter sustained vs
v1's ~195+), and bf16 intermediates keep |rel err| ~1e-6 on the loss.

Tail (~14us serialized, measured via unrolled-tails loop-delta; v1 was
~35us): AllGather the [128, 64] f32 local centroids (8-core AG floor
~5us), reload with core-major packing, cast/pack to bf16, then the
pairwise phase entirely in bf16: -2*X^T*Y cross matmuls + K=64 aug
matmul adding (n_i + n_j) via rows {0,32}, ACT sqrt/relu hinge + Square
with accum_out (NOT DVE tensor_tensor_reduce -- that instruction hangs
the device: NRT_EXEC_UNIT_UNRECOVERABLE), weighted combine, ones-matmul
partition fold, scalar out.
"""

import numpy as np
from contextlib import ExitStack

import concourse.bacc as bacc
import concourse.bass as bass
import concourse.mybir as mybir
import concourse.tile as tile
from concourse.bass_utils import run_bass_kernel_spmd
from concourse.masks import make_identity
from concourse.tile import add_dep_helper

F32 = mybir.dt.float32
BF16 = mybir.dt.bfloat16
AX = mybir.AxisListType
ALU = mybir.AluOpType
ACTF = mybir.ActivationFunctionType

MARGIN = 10.0
ALPHA = 3.0
BETA = 0.3
GAMMA = 0.3

B, S, D = 256, 2048, 128
N_CORES = 8

BPD = 1       # batch items per DMA
BUFS = 8      # big-tile pool buffers
N_DMA_ENG = 2  # HWDGE rings: nc.sync + nc.scalar (gpsimd SWDGE measured slower)


def _emit_body(tc, nc, e, v, out, loc_cent, gath, loc_e, loc_v, gath_e,
               gath_v, n_cores, *, bpd, bufs,
               n_dma_eng, loop_r, bulk, tail, solo, unroll_k, tails_only, ctx):
    chunked = (tail == "full" and loop_r == 1 and unroll_k == 1
               and not solo and bulk != "nored" and bpd == 1)
    Bl = B // n_cores          # local batches per tensor (32)
    nloc = 2 * Bl              # local batch columns (64)
    n_tiles = nloc // bpd
    n_pairs = B * (B - 1) // 2
    w_ev = ALPHA / (B * B)
    w_ee = BETA / (2.0 * n_pairs)
    w_vv = GAMMA / (2.0 * n_pairs)

    consts = ctx.enter_context(tc.tile_pool(name="consts", bufs=1))
    ones_f32 = nc.const_aps.aps[(F32, 1.0)]
    ones_bf16 = nc.const_aps.aps[(BF16, 1.0)]
    b_eps = consts.tile([128, 1], F32, name="b_eps")
    nc.vector.memset(b_eps[:], 1e-12)
    b_margin = consts.tile([128, 1], F32, name="b_margin")
    nc.vector.memset(b_margin[:], MARGIN)
    # one-hot window: W[:, 128] = 1, else 0.  lhsT = W[:, 128-c : 192-c]
    W = consts.tile([128, 256], BF16, name="Wonehot")
    nc.vector.memset(W[:], 0.0)
    nc.vector.memset(W[:, 128:129], 1.0)
    ident = consts.tile([128, 128], F32, name="ident")
    make_identity(nc, ident[:])
    centS = consts.tile([D, nloc], F32, name="centS")

    big = ctx.enter_context(tc.tile_pool(name="big", bufs=bufs))
    half = ctx.enter_context(tc.tile_pool(name="half", bufs=bufs))
    cps = ctx.enter_context(tc.tile_pool(name="cps", bufs=1, space="PSUM"))
    sp = ctx.enter_context(tc.tile_pool(name="sp", bufs=1))
    pps = ctx.enter_context(tc.tile_pool(name="pps", bufs=1, space="PSUM"))
    trash = ctx.enter_context(tc.tile_pool(name="trash", bufs=2))

    T01e = cps.tile([128, 1024], F32, name="T01e")
    T01v = cps.tile([128, 1024], F32, name="T01v")
    Tp = cps.tile([128, 512], F32, name="Tp")

    dma_engines = [nc.sync, nc.scalar, nc.gpsimd][:n_dma_eng]

    def emit_bulk():
        first_dmas = []
        dma_i = 0
        for t in range(n_tiles):
            gi0 = t * bpd
            t_idx, b0 = divmod(gi0, Bl)
            src = (e, v)[t_idx]
            Tb = big.tile([128, bpd * 2048], F32, name="Tb")
            eng = dma_engines[dma_i % len(dma_engines)]
            dma_i += 1
            if bpd == 1:
                dma = eng.dma_start(
                    out=Tb[:],
                    in_=src[b0].rearrange("(p j) d -> p (j d)", p=128),
                )
            else:
                dma = eng.dma_start(
                    out=Tb[:].rearrange("p (b x) -> p b x", b=bpd),
                    in_=src[b0:b0 + bpd].rearrange(
                        "b (p j) d -> p b (j d)", p=128
                    ),
                )
            if t < len(dma_engines):
                first_dmas.append(dma)
            if bulk == "nored":
                continue
            Th = half.tile([128, bpd * 1024], BF16, name="Th")
            if bpd == 1:
                nc.vector.tensor_add(Th[:], Tb[:, 0:1024], Tb[:, 1024:2048])
            else:
                Tb3 = Tb[:].rearrange("p (b x) -> p b x", b=bpd)
                Th3 = Th[:].rearrange("p (b x) -> p b x", b=bpd)
                nc.vector.tensor_add(
                    Th3[:, :, :], Tb3[:, :, 0:1024], Tb3[:, :, 1024:2048]
                )
            for bi in range(bpd):
                c = gi0 + bi
                cm = c % Bl
                T01 = T01e if c < Bl else T01v
                lhsT = W[:, 128 - cm:160 - cm]
                off = bi * 1024
                first = cm == 0
                last = cm == Bl - 1
                nc.tensor.matmul(
                    out=T01[0:32, 0:512], lhsT=lhsT, rhs=Th[:, off:off + 512],
                    start=first, stop=last, skip_group_check=True,
                )
                nc.tensor.matmul(
                    out=T01[0:32, 512:1024], lhsT=lhsT,
                    rhs=Th[:, off + 512:off + 1024],
                    start=first, stop=last, skip_group_check=True,
                )
            if chunked and gi0 + bpd == Bl:
                half_finish("e")
        if bulk == "nored":
            return first_dmas
        if not chunked:
            for h in ("e", "v"):
                finish_only(h)
        return first_dmas

    Cs = sp.tile([32, 128], F32, name="Cs", tag="Cs", bufs=2)
    Cb = sp.tile([D, 2 * B], BF16, name="Cb")
    Cm2 = sp.tile([D, 2 * B], BF16, name="Cm2")
    sqb = sp.tile([D, 2 * B], BF16, name="sqb")
    psn_t = pps.tile([128, 512], F32, name="psn", tag="psn", bufs=1)
    A_e = sp.tile([64, B], BF16, name="A_e")
    A_v = sp.tile([64, B], BF16, name="A_v")
    R_e = sp.tile([64, B], BF16, name="R_e")
    R_v = sp.tile([64, B], BF16, name="R_v")
    acc = sp.tile([128, 6], F32, name="acc")

    def finish_only(h):
        # reduce + transpose + scale one tensor-half into centS cols
        T01 = T01e if h == "e" else T01v
        lo = 0 if h == "e" else Bl
        Csh = sp.tile([32, 128], F32, name="Cs", tag="Cs", bufs=2)
        nc.vector.tensor_reduce(
            out=Csh[:], in_=T01[0:32, :].rearrange("p (j d) -> p d j", d=128),
            axis=AX.X, op=ALU.add,
        )
        nc.tensor.transpose(Tp[:, lo:lo + Bl], Csh[:], ident[0:32, 0:32])
        nc.scalar.mul(centS[:, lo:lo + Bl], Tp[:, lo:lo + Bl], 1.0 / S)

    def pair_block(m2, Ct, Ag, Rt, blk, ci, is_ev):
        P_t = pps.tile([128, 512], F32, name="P", tag="P", bufs=2)
        P_ = P_t[:, 0:256]
        nc.tensor.matmul(
            out=P_, lhsT=m2[:, 128 * blk:128 * blk + 128],
            rhs=Ct, start=True, stop=False, skip_group_check=True,
        )
        nc.tensor.matmul(
            out=P_, lhsT=Ag[:, 128 * blk:128 * blk + 128], rhs=Rt[:],
            start=False, stop=True, skip_group_check=True,
        )
        if is_ev:
            dist = trash.tile([128, B], F32, name="dist", tag="dist")
            hin = trash.tile([128, B], F32, name="hin", tag="hin")
            hsq = trash.tile([128, B], F32, name="hsq", tag="hsq")
            nc.scalar.activation(dist[:], P_, ACTF.Sqrt, bias=b_eps[:])
            nc.scalar.activation(
                hin[:], dist[:], ACTF.Relu, bias=b_margin[:], scale=-1.0,
            )
            nc.scalar.activation(
                hsq[:], hin[:], ACTF.Square, accum_out=acc[:, ci:ci + 1],
            )
        else:
            rel = trash.tile([128, B], F32, name="rel", tag="dist")
            nc.scalar.activation(
                rel[:], P_, ACTF.Relu, accum_out=acc[:, ci:ci + 1],
            )

    def half_finish(h):
        # finish one tensor-half, AllGather it, reload + pack to bf16, norms
        # + aug tiles; for E also the EE clustering blocks (all of this is
        # hidden under the V half of the DMA stream).  E-side loc/reload DMAs
        # go via gpsimd SWDGE so the HWDGE bulk rings never stall on sems.
        finish_only(h)
        lo = 0 if h == "e" else Bl
        loc = loc_e if h == "e" else loc_v
        gat = gath_e if h == "e" else gath_v
        ld_eng = nc.gpsimd if h == "e" else nc.sync
        ld_eng.dma_start(out=loc[:], in_=centS[:, lo:lo + Bl])
        nc.gpsimd.collective_compute(
            "AllGather",
            ALU.bypass,
            replica_groups=[list(range(n_cores))],
            ins=[loc[:]],
            outs=[gat[:]],
        )
        Ch = Cb[:, lo * n_cores:(lo + Bl) * n_cores]
        Ch2 = Cm2[:, lo * n_cores:(lo + Bl) * n_cores]
        sqh = sqb[:, lo * n_cores:(lo + Bl) * n_cores]
        Craw = sp.tile([D, B], F32, name="Craw", tag="Craw", bufs=2)
        co = Craw[:].rearrange("p (c m) -> p c m", c=n_cores)
        gi_ = gat.rearrange("(c p) m -> p c m", c=n_cores)
        if h == "e":
            nc.gpsimd.dma_start(out=co, in_=gi_)
        else:
            hh = n_cores // 2
            nc.sync.dma_start(out=co[:, 0:hh], in_=gi_[:, 0:hh])
            nc.scalar.dma_start(out=co[:, hh:n_cores], in_=gi_[:, hh:n_cores])
        nc.vector.tensor_copy(Ch, Craw[:])
        nc.vector.tensor_scalar_mul(Ch2, Craw[:], -2.0)
        nc.vector.tensor_mul(sqh, Ch, Ch)
        nc.tensor.matmul(
            out=psn_t[0:1, lo * n_cores:(lo + Bl) * n_cores],
            lhsT=ones_bf16, rhs=sqh, skip_group_check=True,
        )
        nh = psn_t[0:1, lo * n_cores:(lo + Bl) * n_cores]
        Ag, Rt = (A_e, R_e) if h == "e" else (A_v, R_v)
        nc.vector.memset(Ag[:], 0.0)
        nc.vector.tensor_copy(Ag[0:1, :], nh)
        nc.vector.memset(Ag[32:33, :], 1.0)
        nc.vector.memset(Rt[:], 0.0)
        nc.vector.memset(Rt[0:1, :], 1.0)
        nc.vector.tensor_copy(Rt[32:33, :], nh)
        if h == "e":
            nc.vector.memset(acc[:], 0.0)
            CbE, Cm2E = Cb[:, 0:B], Cm2[:, 0:B]
            pair_block(Cm2E, CbE, A_e, R_e, 0, 2, False)
            pair_block(Cm2E, CbE, A_e, R_e, 1, 3, False)

    def emit_tail(k):
        if tail == "none":
            fin0 = sp.tile([1, 1], F32, name="fin0", tag="fin0")
            nc.vector.memset(fin0[:], 0.0)
            nc.sync.dma_start(out=loc_cent[:], in_=centS[:])
            return nc.sync.dma_start(out=out[:], in_=fin0[:])
        if not chunked:
            half_finish("e")
        half_finish("v")
        CbE, CbV = Cb[:, 0:B], Cb[:, B:2 * B]
        Cm2E, Cm2V = Cm2[:, 0:B], Cm2[:, B:2 * B]
        pair_block(Cm2E, CbV, A_e, R_v, 0, 0, True)
        pair_block(Cm2E, CbV, A_e, R_v, 1, 1, True)
        pair_block(Cm2V, CbV, A_v, R_v, 0, 4, False)
        pair_block(Cm2V, CbV, A_v, R_v, 1, 5, False)
        # combine: tot = w_ev*(acc0+acc1) + w_ee*(acc2+acc3) + w_vv*(acc4+acc5)
        t_ev = sp.tile([128, 1], F32, name="t_ev", tag="t_ev")
        t_ee = sp.tile([128, 1], F32, name="t_ee", tag="t_ee")
        t_vv = sp.tile([128, 1], F32, name="t_vv", tag="t_vv")
        tot = sp.tile([128, 1], F32, name="tot", tag="tot")
        nc.vector.tensor_add(t_ev[:], acc[:, 0:1], acc[:, 1:2])
        nc.vector.tensor_add(t_ee[:], acc[:, 2:3], acc[:, 3:4])
        nc.vector.tensor_add(t_vv[:], acc[:, 4:5], acc[:, 5:6])
        nc.vector.tensor_scalar_mul(tot[:], t_ev[:], w_ev)
        nc.vector.scalar_tensor_tensor(
            tot[:], t_ee[:], w_ee, tot[:], op0=ALU.mult, op1=ALU.add,
        )
        nc.vector.scalar_tensor_tensor(
            tot[:], t_vv[:], w_vv, tot[:], op0=ALU.mult, op1=ALU.add,
        )
        psF_t = pps.tile([128, 512], F32, name="psF", tag="psn", bufs=1)
        nc.tensor.matmul(out=psF_t[0:1, 0:1], lhsT=ones_f32, rhs=tot[:],
                         skip_group_check=True)
        fin = sp.tile([1, 1], F32, name="fin", tag="fin")
        nc.scalar.copy(fin[:], psF_t[0:1, 0:1])
        od = nc.sync.dma_start(out=out[:], in_=fin[:])
        return od

    if unroll_k > 1 and tails_only:
        emit_bulk()
        for k in range(unroll_k):
            emit_tail(k)
    elif unroll_k > 1:
        prev = None
        for k in range(unroll_k):
            fds = emit_bulk()
            if prev is not None:
                for d in fds:
                    add_dep_helper(d.ins, prev.ins, sync=True,
                                   reason="serialize e2e iterations")
            prev = emit_tail(k)
    else:
        if loop_r > 1:
            with tc.For_i(0, loop_r, 1) as _i:
                emit_bulk()
        else:
            emit_bulk()
        if bulk == "nored":
            nc.vector.memset(centS[:], 0.0)
        emit_tail(0)


def build_nc(n_cores=N_CORES, *, bpd=BPD, bufs=BUFS, n_dma_eng=N_DMA_ENG,
             loop_r=1, bulk="pemov", tail="full", solo=False, unroll_k=1,
             tails_only=False):
    Bl = B // n_cores
    nc = bacc.Bacc("TRN2", num_devices=n_cores)
    e = nc.dram_tensor("expert_concepts", [Bl, S, D], F32,
                       kind="ExternalInput").ap()
    v = nc.dram_tensor("violator_concepts", [Bl, S, D], F32,
                       kind="ExternalInput").ap()
    out = nc.dram_tensor("out", [1, 1], F32, kind="ExternalOutput").ap()
    loc_cent = nc.dram_tensor("loc_cent", [D, 2 * Bl], F32).ap()
    gath = nc.dram_tensor(
        "gath", [n_cores * D, 2 * Bl], F32,
        addr_space="Local" if solo else "Shared",
    ).ap()
    loc_e = nc.dram_tensor("loc_e", [D, Bl], F32).ap()
    loc_v = nc.dram_tensor("loc_v", [D, Bl], F32).ap()
    gath_e = nc.dram_tensor("gath_e", [n_cores * D, Bl], F32,
                            addr_space="Shared").ap()
    gath_v = nc.dram_tensor("gath_v", [n_cores * D, Bl], F32,
                            addr_space="Shared").ap()
    with tile.TileContext(nc) as tc:
        with ExitStack() as ctx:
            _emit_body(tc, nc, e, v, out, loc_cent, gath, loc_e, loc_v,
                       gath_e, gath_v, n_cores,
                       bpd=bpd, bufs=bufs, n_dma_eng=n_dma_eng, loop_r=loop_r,
                       bulk=bulk, tail=tail, solo=solo, unroll_k=unroll_k,
                       tails_only=tails_only, ctx=ctx)
    nc.compile()
    return nc


def make_in_maps(expert_concepts, violator_concepts, n_cores=N_CORES):
    expert_concepts = np.ascontiguousarray(expert_concepts, dtype=np.float32)
    violator_concepts = np.ascontiguousarray(violator_concepts, dtype=np.float32)
    Bl = B // n_cores
    return [
        {
            "expert_concepts": expert_concepts[c * Bl:(c + 1) * Bl],
            "violator_concepts": violator_concepts[c * Bl:(c + 1) * Bl],
        }
        for c in range(n_cores)
    ]


def _run(expert_concepts, violator_concepts, **build_kwargs):
    assert expert_concepts.shape == (B, S, D)
    assert violator_concepts.shape == (B, S, D)
    nc = build_nc(**build_kwargs)
    in_maps = make_in_maps(expert_concepts, violator_concepts)
    res = run_bass_kernel_spmd(nc, in_maps, list(range(N_CORES)))
    return np.float32(res.results[0]["out"][0, 0]), res


def kernel(expert_concepts: np.ndarray, violator_concepts: np.ndarray) -> np.ndarray:
    out, _ = _run(expert_concepts, violator_concepts)
    return out
